# revision 12
# baseline (speedup 1.0000x reference)
"""Trainium2 Bass kernel for ConvTemporalGraphical (gnn_message_passing).

Reference computation (fp32):
    y   = einsum('nctv,oc->notv', x, W) + b        # 1x1 conv channel mix
    out = einsum('nkctv,kvw->nctw', y.reshape(n,K,C,t,v), A)

Shapes: x [16,128,256,64] f32, A [3,64,64], W [384,128], b [384].

Strategy (8 NeuronCores, data-parallel over N, 2 samples per core):
  The two contractions are reordered as
      Z_k[ci,t,w] = sum_v x[ci,t,v] * A[k,v,w]          (graph mixing first)
      out[c,t,w]  = sum_k sum_ci W[(k,c),ci] * Z_k[ci,t,w] + bias2[c,w]
  with bias2[c,w] = sum_{k,v} b[(k,c)] A[k,v,w] (host-precomputed).

  x is shipped to the device as bf16 and loaded via the DMA XBAR
  transpose (one InstDmaTransposeAnt per 32-t chunk) directly into the
  [(2t,64v), pair, ci] layout Step A needs - no PE transposes, no
  PSUM->SBUF transpose drains.  Per (n, 8-t group):
    1. Step A matmul (bf16, FD=384): lhsT=xt pair, rhs=MA where MA
       [128,384] is block-diag([Acat, Acat]), Acat[v,(k w)]=A[k,v,w].
       Two pair-outputs share a 2-bank PSUM tile; drains alternate
       DVE / ACT and convert Z to bf16 in a [ci, 8, 3, 64] buffer.
    2. Step B matmul (bf16, FD=512): accumulate over k in PSUM:
       lhsT=Wt[:,k,:] ([ci,c] bf16), rhs=Z[:, :, k, :] (strided).
    3. gpsimd drains with fused bias add (f32 + PSUM f32 -> bf16) into
       a per-chunk [c, 32, 64] tile; ACT-dispatched DMA stores bf16.
  Output returns to the host as bf16 and is widened to f32 there
  (~0.4% worst-case rounding, tolerance is 2e-2).

kernel(**inputs) shards on host, runs the SPMD program on cores 0-7, and
concatenates the per-core outputs.
"""

import numpy as np
import ml_dtypes

import concourse.bass as bass
import concourse.mybir as mybir
from concourse import bacc
from concourse.bass_utils import run_bass_kernel_spmd
from concourse.tile import TileContext

F32 = mybir.dt.float32
BF16 = mybir.dt.bfloat16

N, C_IN, C_OUT, K, T, V = 16, 128, 128, 3, 256, 64
N_CORES = 8
N_PER_CORE = N // N_CORES  # 2
TC = 32                    # t-chunk size (DMA granularity in and out)
N_CHUNKS = T // TC         # 8
QG = TC // 8               # 4 groups (8 t's = 4 pairs) per chunk
DMAT_LEAD = 3              # chunks of lead for the transposing loads
B_LAG = 3                  # groups of lag between step A and step B


def build(reps: int = 1):
    nc = bacc.Bacc(
        "TRN2", target_bir_lowering=False, debug=False, num_devices=N_CORES
    )
    xs = nc.dram_tensor("xs", [N_PER_CORE, C_IN, T, V], BF16, kind="ExternalInput")
    wt = nc.dram_tensor("wt", [C_IN, K, C_OUT], BF16, kind="ExternalInput")
    ma = nc.dram_tensor("ma", [128, 2, K, V], BF16, kind="ExternalInput")
    bias2r = nc.dram_tensor("bias2r", [C_OUT, 8, V], F32, kind="ExternalInput")
    out = nc.dram_tensor(
        "out", [N_PER_CORE, C_OUT, T, V], BF16, kind="ExternalOutput"
    )

    with TileContext(nc) as tc:
        with (
            tc.tile_pool(name="const", bufs=1) as cpool,
            tc.tile_pool(name="xt", bufs=4) as xtpool,
            tc.tile_pool(name="z", bufs=5) as zpool,
            tc.tile_pool(name="o", bufs=3) as opool,
            tc.tile_pool(name="ps_z", bufs=3, space="PSUM") as ps_z,
            tc.tile_pool(name="ps_o", bufs=2, space="PSUM") as ps_o,
        ):
            # consts on the gpsimd DMA queue so the sync queue's first
            # transposing load issues immediately
            ma_sb = cpool.tile([128, 2, K, V], BF16, tag="ma")
            nc.gpsimd.dma_start(out=ma_sb[:], in_=ma[:])
            wt_sb = cpool.tile([C_IN, K, C_OUT], BF16, tag="wt")
            nc.gpsimd.dma_start(out=wt_sb[:], in_=wt[:])
            bias_sb = cpool.tile([C_OUT, 8, V], F32, tag="bias")
            nc.gpsimd.dma_start(out=bias_sb[:], in_=bias2r[:])

            for _ in range(reps):
                groups = [
                    (n, c, q)
                    for n in range(N_PER_CORE)
                    for c in range(N_CHUNKS)
                    for q in range(QG)
                ]
                chunks = [(n, c) for n in range(N_PER_CORE) for c in range(N_CHUNKS)]
                st = {}

                def chunk_state(n, c):
                    if (n, c) not in st:
                        st[(n, c)] = {
                            "o": opool.tile(
                                [C_OUT, TC, V], BF16, tag="o", name="o_sb"
                            ),
                            "xt": None,
                        }
                    return st[(n, c)]

                def stage_dmat(n, c):
                    # one XBAR transposing load per 32-t chunk:
                    # in  [ci=128, (32t 64v)=2048] bf16 (contiguous rows)
                    # out [p=(2t 64v), pair=16, ci=128]
                    s = chunk_state(n, c)
                    xt_sb = xtpool.tile([128, TC // 2, 128], BF16, tag="xt")
                    nc.sync.dma_start(
                        out=xt_sb[:],
                        in_=xs[n, :, c * TC : (c + 1) * TC, :],
                        transpose=True,
                    )
                    s["xt"] = xt_sb

                def stage_a(n, c, q):
                    # Elementwise budget per group: z drains 2x768 + bias-add
                    # 512, vs a ~1.56us PE period.  The bias-add (tensor_tensor
                    # on PSUM) is DVE-only, so give ACT both z halves on even
                    # groups and split them on odd groups: DVE ~1.2us,
                    # ACT ~1.35us per group.
                    s = chunk_state(n, c)
                    xt_sb = s["xt"]
                    z = zpool.tile([C_IN, 8, K, V], BF16, tag="z", name="z_sb")
                    st[(n, c, q)] = z
                    gi = (n * N_CHUNKS + c) * QG + q
                    for h in range(2):
                        z_ps = ps_z.tile([C_IN, 2, 512], F32, tag="zp")
                        for jj in range(2):
                            nc.tensor.matmul(
                                z_ps[:, jj, 0 : 2 * K * V],
                                xt_sb[:, 4 * q + 2 * h + jj, :],
                                ma_sb[:],
                                start=True,
                                stop=True,
                            )
                        if h == 0 and gi % 2 == 1:
                            nc.vector.tensor_copy(
                                out=z[:, 4 * h : 4 * h + 4, :, :],
                                in_=z_ps[:, :, 0 : 2 * K * V],
                            )
                        else:
                            nc.scalar.copy(
                                out=z[:, 4 * h : 4 * h + 4, :, :],
                                in_=z_ps[:, :, 0 : 2 * K * V],
                            )

                def stage_b(n, c, q):
                    s = chunk_state(n, c)
                    z = st.pop((n, c, q))
                    o_ps = ps_o.tile([C_OUT, 8, V], F32, tag="op")
                    for k in range(K):
                        nc.tensor.matmul(
                            o_ps[:],
                            wt_sb[:, k, :],
                            z[:, :, k, :],
                            start=(k == 0),
                            stop=(k == K - 1),
                        )
                    # fused bias-add drain on DVE (tensor_tensor with a PSUM
                    # operand can only run there)
                    nc.vector.tensor_add(
                        out=s["o"][:, 8 * q : 8 * (q + 1), :],
                        in0=o_ps[:],
                        in1=bias_sb[:],
                    )
                    # out DMA dispatched from the SP queue: it blocks on the
                    # gpsimd bias add, and SP is the only queue with slack to
                    # absorb that wait without convoying the drains
                    last_chunk = n == N_PER_CORE - 1 and c == N_CHUNKS - 1
                    if last_chunk:
                        # epilogue: stream each group out as soon as its bias
                        # add lands, so the final post-compute DMA is 1KB/
                        # partition instead of 4KB
                        t0 = c * TC + 8 * q
                        nc.sync.dma_start(
                            out=out[n, :, t0 : t0 + 8, :],
                            in_=s["o"][:, 8 * q : 8 * (q + 1), :],
                        )
                        if q == QG - 1:
                            del st[(n, c)]
                    elif q == QG - 1:
                        nc.sync.dma_start(
                            out=out[n, :, c * TC : (c + 1) * TC, :],
                            in_=s["o"][:],
                        )
                        del st[(n, c)]

                # prologue: transposing loads for the first chunks
                for ci_ in range(DMAT_LEAD):
                    stage_dmat(*chunks[ci_])

                for i in range(len(groups) + B_LAG):
                    if i < len(groups):
                        n, c, q = groups[i]
                        if q == 0:
                            ci_ = chunks.index((n, c)) + DMAT_LEAD
                            if ci_ < len(chunks):
                                stage_dmat(*chunks[ci_])
                        stage_a(n, c, q)
                    if i >= B_LAG:
                        stage_b(*groups[i - B_LAG])

    nc.compile()
    return nc


def prep_weights(A, W, b):
    A = np.asarray(A, np.float32)
    W = np.asarray(W, np.float32)
    b = np.asarray(b, np.float32)
    wt = np.ascontiguousarray(
        W.reshape(K, C_OUT, C_IN).transpose(2, 0, 1)
    ).astype(ml_dtypes.bfloat16)  # [ci, k, c]
    acat = np.ascontiguousarray(A.transpose(1, 0, 2))  # [v, k, w]
    ma = np.zeros((128, 2, K, V), np.float32)
    ma[0:64, 0] = acat
    ma[64:128, 1] = acat
    ma = ma.astype(ml_dtypes.bfloat16)
    bias2 = np.einsum("kc,kw->cw", b.reshape(K, C_OUT), A.sum(axis=1))
    bias2r = np.ascontiguousarray(
        np.broadcast_to(bias2[:, None, :], (C_OUT, 8, V))
    ).astype(np.float32)
    return wt, ma, bias2r


_NC_CACHE = {}


def get_nc(reps: int = 1):
    if reps not in _NC_CACHE:
        _NC_CACHE[reps] = build(reps)
    return _NC_CACHE[reps]


def make_in_maps(x, A, W, b):
    x = np.asarray(x, np.float32).astype(ml_dtypes.bfloat16)
    wt, ma, bias2r = prep_weights(A, W, b)
    return [
        {
            "xs": np.ascontiguousarray(x[i * N_PER_CORE : (i + 1) * N_PER_CORE]),
            "wt": wt,
            "ma": ma,
            "bias2r": bias2r,
        }
        for i in range(N_CORES)
    ]


def run(x, A, W, b, reps: int = 1):
    nc = get_nc(reps)
    in_maps = make_in_maps(x, A, W, b)
    res = run_bass_kernel_spmd(nc, in_maps, list(range(N_CORES)))
    return np.concatenate(
        [
            np.asarray(res.results[i]["out"]).astype(np.float32)
            for i in range(N_CORES)
        ],
        axis=0,
    )


def kernel(x, A, W, b):
    return run(x, A, W, b, reps=1)
